# revision 1
# baseline (speedup 1.0000x reference)
"""Multi-head attention (B=2, S=2048, D=1024, 16 heads x 64) on 8 TRN2 cores.

Sharding: tensor-parallel over heads. Core c owns heads {2c, 2c+1} =
rows [128c, 128c+128) of Wq/Wk/Wv, computes its (B, S, 128) slice of the
context, host concatenates along the feature axis. No collectives.

Per-core pipeline (matmul operands bf16, f32 PSUM accumulation):
  x, W: f32 HWDGE load -> DVE cast to bf16 -> PE transpose (1 cyc/row,
  8 chunks packed per PSUM bank) -> DVE copy to xT/wT.
  qT/kT/vT projections (+bias per-partition). v re-transposed to [t, w]
  on PE. mask -> em[t] = exp(-1e4*(1-mask[t])) folded into V rows
  (exact: exp(a+b) = exp(a)exp(b)); V carries an extra em column so the
  PV matmul also produces the softmax denominator Z.
  scoresT[t,s] = k[t].q[s], two key-chunks per 2-bank PSUM tile -> one
  ACT exp (scale=1/8, [128,1024]) straight from PSUM -> PV accumulate
  (65 x 512), software-pipelined one pair behind QK so the PE queue
  never head-of-line-blocks the next QK behind a PV waiting on exp ->
  PE transpose -> scale by 1/Z -> out (output DMA on GPSIMD/SWDGE to
  keep the HWDGE queues free).
"""

import sys

if "/opt/trn_rl_repo" not in sys.path:
    sys.path.insert(0, "/opt/trn_rl_repo")

import numpy as np

B = 2
S = 2048
D = 1024
NCORES = 8
WC = 128          # per-core projection width (2 heads x 64)
HEADS = 2         # heads per core
W = 64            # head dim
KC = D // 128     # contraction chunks (8)
SC = S // 128     # 128-row chunks of S (16)
SEG = 512         # matmul moving-dim segment
NSEG = S // SEG   # 4
SBLK = 512        # attention s-block
NBLK = S // SBLK  # 4


def _build():
    import concourse.bass as bass
    import concourse.tile as tile
    from concourse import bacc, mybir
    from concourse.masks import make_identity

    f32 = mybir.dt.float32
    bf16 = mybir.dt.bfloat16
    EXP = mybir.ActivationFunctionType.Exp

    nc = bacc.Bacc("TRN2", target_bir_lowering=False, debug=False)

    x_d = nc.dram_tensor("hidden_states", [B, S, D], f32, kind="ExternalInput")
    m_d = nc.dram_tensor("attn_mask", [B, S], f32, kind="ExternalInput")
    wq_d = nc.dram_tensor("wq", [WC, D], f32, kind="ExternalInput")
    wk_d = nc.dram_tensor("wk", [WC, D], f32, kind="ExternalInput")
    wv_d = nc.dram_tensor("wv", [WC, D], f32, kind="ExternalInput")
    bq_d = nc.dram_tensor("bq", [WC], f32, kind="ExternalInput")
    bk_d = nc.dram_tensor("bk", [WC], f32, kind="ExternalInput")
    bv_d = nc.dram_tensor("bv", [WC], f32, kind="ExternalInput")
    o_d = nc.dram_tensor("out", [B, S, WC], f32, kind="ExternalOutput")

    with tile.TileContext(nc) as tc:
        consts = tc.alloc_tile_pool(name="consts", bufs=1)
        xp = tc.alloc_tile_pool(name="xp", bufs=5)
        xbp = tc.alloc_tile_pool(name="xbp", bufs=6)
        xtp = tc.alloc_tile_pool(name="xtp", bufs=2)
        qkp = tc.alloc_tile_pool(name="qkp", bufs=2)
        vp = tc.alloc_tile_pool(name="vp", bufs=2)
        etp = tc.alloc_tile_pool(name="etp", bufs=6)
        hp = tc.alloc_tile_pool(name="hp", bufs=4)
        op = tc.alloc_tile_pool(name="op", bufs=8)
        ps_work = tc.alloc_tile_pool(name="ps_work", bufs=1, space="PSUM")
        ps_tr = tc.alloc_tile_pool(name="ps_tr", bufs=2, space="PSUM")
        ps_sc = tc.alloc_tile_pool(name="ps_sc", bufs=2, space="PSUM")
        ps_h = tc.alloc_tile_pool(name="ps_h", bufs=1, space="PSUM")

        ident = consts.tile([128, 128], f32, tag="ident")
        make_identity(nc, ident[:, :])
        identb = consts.tile([128, 128], bf16, tag="identb")
        make_identity(nc, identb[:, :])

        CPY = mybir.ActivationFunctionType.Copy

        def transpose4(dst_slices, src, chunks, tag="tr", copy_eng="vector"):
            """PE-transpose `chunks` 128x128 bf16 blocks of `src`, packed 8
            per PSUM bank, one copy per pack into dst_slices(kc0, n). The
            copy engine is DVE by default; ACT during the b0 prep phase
            (where the ScalarEngine is otherwise idle) to unbind DVE."""
            for kc0 in range(0, chunks, 8):
                n = min(8, chunks - kc0)
                ptf = ps_tr.tile([128, 512], f32, tag=tag, name="trp")
                pt = ptf[:, :].bitcast(bf16).rearrange("p (a b) -> p a b", b=128)
                for j in range(n):
                    nc.tensor.transpose(
                        pt[:, j, :],
                        src[:, (kc0 + j) * 128:(kc0 + j + 1) * 128],
                        identb[:, :],
                    )
                nc.vector.tensor_copy(dst_slices(kc0, n), pt[:, 0:n, :])

        # --- weights: f32 load, DVE cast bf16, PE transpose to [d, w] ---
        wts = {}
        for name, wd in (("q", wq_d), ("k", wk_d), ("v", wv_d)):
            wf = xp.tile([128, D], f32, tag="xf")
            nc.scalar.dma_start(out=wf[:, :], in_=wd[:, :])
            wb = xbp.tile([128, D], bf16, tag="x")
            nc.vector.tensor_copy(wb[:, :], wf[:, :])
            wt = consts.tile([128, KC, 128], bf16, tag=f"wt_{name}")
            transpose4(lambda kc0, n, wt=wt: wt[:, kc0:kc0 + n, :], wb, KC)
            wts[name] = wt

        bias = {}
        for name, bd in (("q", bq_d), ("k", bk_d), ("v", bv_d)):
            bc = consts.tile([128, 1], f32, tag=f"b_{name}")
            nc.gpsimd.dma_start(
                out=bc[:, :], in_=bd.ap().rearrange("(p one) -> p one", one=1)
            )
            bias[name] = bc

        # --- mask -> em[t] = exp(1e4*m - 1e4), laid out [t_local, t_chunk] ---
        mb = consts.tile([128, 1], f32, tag="mbias")
        nc.vector.memset(mb[:, :], -10000.0)
        ems = []
        for b in range(B):
            msk = consts.tile([128, SC], f32, tag=f"mask{b}")
            nc.gpsimd.dma_start(
                out=msk[:, :], in_=m_d[b].rearrange("(c p) -> p c", p=128)
            )
            em = consts.tile([128, SC], f32, tag=f"em{b}")
            nc.scalar.activation(em[:, :], msk[:, :], EXP, scale=10000.0, bias=mb[:, :])
            ems.append(em)

        for b in range(B):
            # --- xT[d, s] bf16: f32 load, DVE cast, PE transpose ---
            xt = xtp.tile([128, KC, S], bf16, tag="xt")
            for sc in range(SC):
                xf = xp.tile([128, D], f32, tag="xf")
                nc.sync.dma_start(out=xf[:, :], in_=x_d[b, sc * 128:(sc + 1) * 128, :])
                xb = xbp.tile([128, D], bf16, tag="x")
                nc.vector.tensor_copy(xb[:, :], xf[:, :])
                transpose4(
                    lambda kc0, n, sc=sc: xt[:, kc0:kc0 + n, sc * 128:(sc + 1) * 128],
                    xb, KC,
                )

            # --- projections: qT/kT/vT [w, s] = W.T-chunks @ xT ---
            qt = qkp.tile([128, S], bf16, tag="qt")
            kt = qkp.tile([128, S], bf16, tag="kt")
            vt = qkp.tile([128, S], bf16, tag="vt")
            for dst, wname in ((qt, "q"), (kt, "k"), (vt, "v")):
                wt = wts[wname]
                for sg in range(NSEG):
                    pp = ps_work.tile([128, SEG], f32, tag="work")
                    for kc in range(KC):
                        nc.tensor.matmul(
                            pp[:, :],
                            lhsT=wt[:, kc, :],
                            rhs=xt[:, kc, sg * SEG:(sg + 1) * SEG],
                            start=(kc == 0),
                            stop=(kc == KC - 1),
                        )
                    nc.vector.tensor_scalar_add(
                        dst[:, sg * SEG:(sg + 1) * SEG], pp[:, :], bias[wname][:, :]
                    )

            # --- v'' [t, (head, 65)]: PE transpose vt chunk, em scale, em col ---
            v2 = vp.tile([128, SC, HEADS, W + 1], bf16, tag="v2")
            for scc in range(SC):
                pvf = ps_tr.tile([128, 256], f32, tag="tr", name="trv")
                pv = pvf[:, :].bitcast(bf16).rearrange("p (a b) -> p a b", b=128)
                nc.tensor.transpose(
                    pv[:, 0, :], vt[:, scc * 128:(scc + 1) * 128], identb[:, :]
                )
                nc.vector.tensor_scalar(
                    out=v2[:, scc, :, 0:W],
                    in0=pv[:, 0, :].rearrange("p (h w) -> p h w", h=HEADS),
                    scalar1=ems[b][:, scc:scc + 1],
                    scalar2=None,
                    op0=mybir.AluOpType.mult,
                )
                for h in range(HEADS):
                    nc.vector.tensor_copy(
                        v2[:, scc, h, W:W + 1], ems[b][:, scc:scc + 1]
                    )

            # --- attention: s-block 512, two t-chunks packed per PSUM tile ---
            for h in range(HEADS):
                for blk in range(NBLK):
                    ph = ps_h.tile([W + 1, SEG], f32, tag="ph")
                    pend = None
                    for tp in range(0, SC, 2):
                        psc = ps_sc.tile([128, 2, SEG], f32, tag="sc")
                        for j in range(2):
                            nc.tensor.matmul(
                                psc[:, j, :],
                                lhsT=kt[h * W:(h + 1) * W,
                                        (tp + j) * 128:(tp + j + 1) * 128],
                                rhs=qt[h * W:(h + 1) * W,
                                       blk * SBLK:(blk + 1) * SBLK],
                                start=True,
                                stop=True,
                            )
                        et = etp.tile([128, 2, SEG], bf16, tag="et")
                        nc.scalar.activation(et[:, :, :], psc[:, :, :], EXP, scale=0.125)
                        # PV of the PREVIOUS pair is emitted after this QK/exp
                        # so the PE queue never head-of-line-blocks the next QK
                        # behind a PV that waits on the current exp.
                        if pend is not None:
                            ptp, pet = pend
                            for j in range(2):
                                nc.tensor.matmul(
                                    ph[:, :],
                                    lhsT=v2[:, ptp + j, h, :],
                                    rhs=pet[:, j, :],
                                    start=(ptp == 0 and j == 0),
                                    stop=False,
                                )
                        pend = (tp, et)
                    ptp, pet = pend
                    for j in range(2):
                        nc.tensor.matmul(
                            ph[:, :],
                            lhsT=v2[:, ptp + j, h, :],
                            rhs=pet[:, j, :],
                            start=False,
                            stop=(j == 1),
                        )
                    hsb = hp.tile([W + 1, SBLK], f32, tag="hsb")
                    nc.vector.tensor_copy(hsb[:, :], ph[:, :])
                    for ss in range(SBLK // 128):
                        pt = ps_tr.tile([128, 512], f32, tag="tr", name="trh")
                        nc.tensor.transpose(
                            pt[:, 0:W + 1],
                            hsb[:, ss * 128:(ss + 1) * 128],
                            ident[0:W + 1, 0:W + 1],
                        )
                        rec = op.tile([128, 1], f32, tag="rec")
                        nc.vector.reciprocal(rec[:, :], pt[:, W:W + 1])
                        ot = op.tile([128, W], f32, tag="ot")
                        nc.vector.tensor_scalar_mul(ot[:, :], pt[:, 0:W], rec[:, :])
                        s0 = blk * SBLK + ss * 128
                        nc.gpsimd.dma_start(
                            out=o_d[b, s0:s0 + 128, h * W:(h + 1) * W], in_=ot[:, :]
                        )

        for p in (ps_h, ps_sc, ps_tr, ps_work, op, hp, etp, vp, qkp, xtp, xbp, xp,
                  consts):
            p.release()

    nc.finalize()
    return nc


_NC = None


def _get_nc():
    global _NC
    if _NC is None:
        _NC = _build()
    return _NC


def _in_maps(inputs):
    x = np.ascontiguousarray(np.asarray(inputs["hidden_states"], dtype=np.float32))
    m = np.ascontiguousarray(np.asarray(inputs["attn_mask"], dtype=np.float32))
    maps = []
    for c in range(NCORES):
        sl = slice(c * WC, (c + 1) * WC)
        maps.append({
            "hidden_states": x,
            "attn_mask": m,
            "wq": np.ascontiguousarray(np.asarray(inputs["Wq"], dtype=np.float32)[sl]),
            "wk": np.ascontiguousarray(np.asarray(inputs["Wk"], dtype=np.float32)[sl]),
            "wv": np.ascontiguousarray(np.asarray(inputs["Wv"], dtype=np.float32)[sl]),
            "bq": np.ascontiguousarray(np.asarray(inputs["bq"], dtype=np.float32)[sl]),
            "bk": np.ascontiguousarray(np.asarray(inputs["bk"], dtype=np.float32)[sl]),
            "bv": np.ascontiguousarray(np.asarray(inputs["bv"], dtype=np.float32)[sl]),
        })
    return maps


def _run(inputs, trace=False):
    from concourse.bass_utils import run_bass_kernel_spmd

    nc = _get_nc()
    res = run_bass_kernel_spmd(
        nc, _in_maps(inputs), core_ids=list(range(NCORES)), trace=trace
    )
    out = np.concatenate([res.results[c]["out"] for c in range(NCORES)], axis=2)
    return np.ascontiguousarray(out, dtype=np.float32), res


def kernel(**inputs):
    out, _ = _run(inputs, trace=False)
    return out



# revision 13
# speedup vs baseline: 1.4949x; 1.4949x over previous
"""Multi-head attention (B=2, S=2048, D=1024, 16 heads x 64) on 8 TRN2 cores.

Sharding: batch x head-group. Core c owns batch b = c//4 and head group
g = c%4 (4 heads, W-rows [256g, 256g+256)). Core output is the (2048, 256)
feature slice; host assembles [B, S, D]. No collectives.

Per-core pipeline (bf16 matmul operands, f32 PSUM):
  x, W: f32 load -> DVE cast bf16 -> PE transpose (packed 8/bank) -> xt/wt.
  q,k proj as [w, s] (lhsT = W.T chunk, rhs = xt). v proj as [s, w]
  (lhsT = xt chunk, rhs = Wv.T) -> v2[t, h, 65] with em[t]-scaled values
  and em[t] in column 64 (em = exp(1e4*mask - 1e4) folds the additive
  mask exactly; the 65th column makes PV also produce the softmax
  denominator Z). Projection biases are zeros by problem spec; skipped.
  Attention: 8 blocks (2 head pairs x 4 s-blocks of 512), software-
  pipelined one block deep: block k runs QK+exp while PV of block k-1
  consumes its stashed et tiles, so ACT (the bottleneck: 1024-row exp =
  ~1.11us, 128 calls = ~143us) never waits on PE.
  QK: two row-tiled K=64 matmuls, tile_position (0,0)/(64,0), run
  concurrently on the PE (measured 1.7x). PV in "swap" form: out[s=128,
  65] = et-chunk.T @ v2[t, 65] at 100% PE utilization (measured 32ns).
  Later q/k projection segments are interleaved into the attention loop
  ("borrows" of a psc PSUM slot) inside the PE slack.
  Finalize: DVE copies ph -> SBUF, GPSIMD divides by Z, HWDGE DMA out.
"""

import sys

if "/opt/trn_rl_repo" not in sys.path:
    sys.path.insert(0, "/opt/trn_rl_repo")

import numpy as np

B = 2
S = 2048
D = 1024
NCORES = 8
WC = 256          # per-core projection width (4 heads x 64)
NH = 4            # heads per core
NP = 2            # head pairs per core
W = 64            # head dim
KC = D // 128     # contraction chunks (8)
SC = S // 128     # 128-row chunks of S (16)
SEG = 512         # proj segment
NSEG = S // SEG   # 4
SBLK = 512        # attention s-block
NBLK = S // SBLK  # 4


def _build():
    import concourse.bass as bass
    import concourse.tile as tile
    from concourse import bacc, mybir
    from concourse.masks import make_identity

    f32 = mybir.dt.float32
    bf16 = mybir.dt.bfloat16
    EXP = mybir.ActivationFunctionType.Exp
    DIV = mybir.AluOpType.divide
    MUL = mybir.AluOpType.mult

    nc = bacc.Bacc("TRN2", target_bir_lowering=False, debug=False)

    x_d = nc.dram_tensor("x", [S, D], f32, kind="ExternalInput")
    m_d = nc.dram_tensor("m", [S], f32, kind="ExternalInput")
    wq_d = nc.dram_tensor("wq", [WC, D], f32, kind="ExternalInput")
    wk_d = nc.dram_tensor("wk", [WC, D], f32, kind="ExternalInput")
    wv_d = nc.dram_tensor("wv", [WC, D], f32, kind="ExternalInput")
    o_d = nc.dram_tensor("out", [S, WC], f32, kind="ExternalOutput")

    with tile.TileContext(nc) as tc:
        consts = tc.alloc_tile_pool(name="consts", bufs=1)
        xfp = tc.alloc_tile_pool(name="xfp", bufs=3)
        xbp = tc.alloc_tile_pool(name="xbp", bufs=3)
        etp = tc.alloc_tile_pool(name="etp", bufs=2 * SC)
        hsp = tc.alloc_tile_pool(name="hsp", bufs=4)
        otp = tc.alloc_tile_pool(name="otp", bufs=4)
        ps_qk = tc.alloc_tile_pool(name="ps_qk", bufs=2, space="PSUM")
        ps_ph = tc.alloc_tile_pool(name="ps_ph", bufs=4, space="PSUM")

        identb = consts.tile([128, 128], bf16, tag="identb")
        make_identity(nc, identb[:, :])

        # persistent SBUF tensors
        xt = consts.tile([128, KC, S], bf16, tag="xt")           # x.T
        wts = {n: consts.tile([128, KC, WC], bf16, tag=f"wt_{n}", name=f"wt_{n}")
               for n in ("q", "k", "v")}
        qt = consts.tile([128, NP, S], bf16, tag="qt")
        kt = consts.tile([128, NP, S], bf16, tag="kt")
        v2 = consts.tile([128, SC, NH, W + 1], bf16, tag="v2")
        em = consts.tile([128, SC], f32, tag="em")

        # --- weights + mask: DMA first ---
        wbufs = {}
        for i, (name, wd) in enumerate((("q", wq_d), ("k", wk_d), ("v", wv_d))):
            for blk in range(2):
                wf = xfp.tile([128, D], f32, tag="wf", name="wf", bufs=6)
                nc.sync.dma_start(out=wf[:, :], in_=wd[blk * 128:(blk + 1) * 128, :])
                wbufs[(name, blk)] = wf

        msk = consts.tile([128, SC], f32, tag="msk")
        nc.gpsimd.dma_start(out=msk[:, :], in_=m_d.ap().rearrange("(c p) -> p c", p=128))
        mb = consts.tile([128, 1], f32, tag="mb")
        nc.vector.memset(mb[:, :], -10000.0)
        # em[t] = exp(1e4*mask - 1e4)  (1 for kept keys, ~0 for masked)
        nc.scalar.activation(em[:, :], msk[:, :], EXP, scale=10000.0, bias=mb[:, :])

        for name in ("q", "k", "v"):
            for blk in range(2):
                wf = wbufs[(name, blk)]
                wb = xbp.tile([128, D], bf16, tag="xb", name="wb")
                nc.vector.tensor_copy(wb[:, :], wf[:, :])
                pt_f = ps_qk.tile([128, 512], f32, tag="psc", name="wtr")
                pt = pt_f[:, :].bitcast(bf16).rearrange("p (a b) -> p a b", b=128)
                for kc in range(KC):
                    nc.tensor.transpose(pt[:, kc, :], wb[:, kc * 128:(kc + 1) * 128],
                                        identb[:, :])
                nc.vector.tensor_copy(wts[name][:, :, blk * 128:(blk + 1) * 128],
                                      pt[:, :, :])

        # v2 Z columns = em (bf16 cast)
        for h in range(NH):
            nc.vector.tensor_copy(
                v2[:, :, h, W:W + 1],
                em[:, :].rearrange("p (c one) -> p c one", one=1))

        # --- x: DMA, cast, PE transpose into xt [d, kc, s] ---
        for sc in range(SC):
            xf = xfp.tile([128, D], f32, tag="xf", name="xf")
            nc.sync.dma_start(out=xf[:, :], in_=x_d[sc * 128:(sc + 1) * 128, :])
            xb = xbp.tile([128, D], bf16, tag="xb", name="xb")
            nc.vector.tensor_copy(xb[:, :], xf[:, :])
            pt_f = ps_qk.tile([128, 512], f32, tag="psc", name="xtr")
            pt = pt_f[:, :].bitcast(bf16).rearrange("p (a b) -> p a b", b=128)
            for kc in range(KC):
                nc.tensor.transpose(pt[:, kc, :], xb[:, kc * 128:(kc + 1) * 128],
                                    identb[:, :])
            nc.vector.tensor_copy(xt[:, :, sc * 128:(sc + 1) * 128], pt[:, :, :])

        def proj_seg(dst, wname, pair, sseg):
            """dst[:, pair, sseg*512:...] = (W.T chunks @ xt) for one segment."""
            pp = ps_qk.tile([128, 512], f32, tag="psc", name="pp")
            wt = wts[wname]
            for kc in range(KC):
                nc.tensor.matmul(
                    pp[:, :],
                    lhsT=wt[:, kc, pair * 128:(pair + 1) * 128],
                    rhs=xt[:, kc, sseg * SEG:(sseg + 1) * SEG],
                    start=(kc == 0), stop=(kc == KC - 1),
                )
            nc.vector.tensor_copy(dst[:, pair, sseg * SEG:(sseg + 1) * SEG], pp[:, :])

        def vproj_sc(sc):
            """v2[:, sc, h, 0:64] = em[sc] * (x @ Wv.T)[sc-chunk] (as [s, w'])."""
            pv = ps_ph.tile([128, 512], f32, tag="ph", name="pv")
            for kc in range(KC):
                nc.tensor.matmul(
                    pv[:, 0:WC],
                    lhsT=xt[:, kc, sc * 128:(sc + 1) * 128],
                    rhs=wts["v"][:, kc, :],
                    start=(kc == 0), stop=(kc == KC - 1),
                )
            nc.vector.tensor_scalar(
                out=v2[:, sc, :, 0:W],
                in0=pv[:, 0:WC].rearrange("p (h w) -> p h w", h=NH),
                scalar1=em[:, sc:sc + 1], scalar2=None, op0=MUL,
            )

        # k (pair 0) + first two q segments before attention starts
        for sseg in range(NSEG):
            proj_seg(kt, "k", 0, sseg)
        proj_seg(qt, "q", 0, 0)
        proj_seg(qt, "q", 0, 1)

        # --- attention: 8 blocks, PV pipelined one block behind ---
        def qk_mms(psc, pair, blk, tcc):
            for j in range(2):
                nc.tensor.matmul(
                    psc[:, j, :],
                    lhsT=kt[j * W:(j + 1) * W, pair, tcc * 128:(tcc + 1) * 128],
                    rhs=qt[j * W:(j + 1) * W, pair, blk * SBLK:(blk + 1) * SBLK],
                    start=True, stop=True,
                )

        def pv_mms(ph, pair, tcc, et):
            # start=False always: a start=True clears the WHOLE bank's
            # has_written bits, wiping the other head's region sharing the
            # bank. The banks are DVE-zeroed in alloc_ph instead; matmuls
            # then initialize-or-accumulate per element correctly.
            for j in range(2):
                h = pair * 2 + j
                for sc4 in range(4):
                    nc.tensor.matmul(
                        ph[sc4][:, j, 0:W + 1],
                        lhsT=et[:, j, sc4 * 128:(sc4 + 1) * 128],
                        rhs=v2[:, tcc, h, :],
                        start=False, stop=(tcc == SC - 1),
                        skip_group_check=True,
                    )

        def alloc_ph():
            # per-head stride 66 f32 (not 65): keeps the two heads'
            # accumulation regions on disjoint 8-byte PSUM cachelines
            ph_f = [ps_ph.tile([128, 512], f32, tag="ph", name="ph")
                    for _ in range(4)]
            for p in ph_f:
                nc.vector.memset(p[:, 0:2 * (W + 2)], 0.0)
            return [p[:, 0:2 * (W + 2)].rearrange("p (h w) -> p h w", w=W + 2)
                    for p in ph_f]

        def finalize(ph, pair, blk):
            # h = ph[:, j, 0:64] / Z, Z = ph[:, j, 64]
            for sc4 in range(4):
                hsb = hsp.tile([128, 2, W + 2], f32, tag="hsb")
                nc.vector.tensor_copy(hsb[:, :, :], ph[sc4][:, :, :])
                rec = otp.tile([128, 2], f32, tag="rec")
                nc.vector.reciprocal(
                    rec[:, :], hsb[:, :, W:W + 1].rearrange("p h one -> p (h one)"))
                ot = otp.tile([128, 2 * W], f32, tag="ot")
                for j in range(2):
                    nc.gpsimd.tensor_scalar(
                        out=ot[:, j * W:(j + 1) * W],
                        in0=hsb[:, j, 0:W],
                        scalar1=rec[:, j:j + 1],
                        scalar2=None, op0=MUL,
                    )
                s0 = blk * SBLK + sc4 * 128
                nc.sync.dma_start(
                    out=o_d[s0:s0 + 128, pair * 128:(pair + 1) * 128],
                    in_=ot[:, :])

        blocks = [(pair, blk) for pair in range(NP) for blk in range(NBLK)]
        # psc-slot borrows per block index (emitted at tcc 5 / 11; block 4's
        # kproj(1,3) at tcc 4 lands just before its tcc-12 QK needs it)
        borrows = {
            1: [("q", 0, 2), ("k", 1, 0)],
            2: [("q", 0, 3), ("k", 1, 1)],
            3: [("q", 1, 0), ("k", 1, 2)],
            4: [("q", 1, 1), ("k", 1, 3)],
            5: [("q", 1, 2)],
            6: [("q", 1, 3)],
        }
        prev = None
        for bi, (pair, blk) in enumerate(blocks):
            ph = alloc_ph() if prev is not None else None
            ets = []
            for tcc in range(SC):
                if bi == 0:
                    vproj_sc(tcc)
                psc = ps_qk.tile([128, 2, 512], f32, tag="psc", name="psc")
                qk_mms(psc, pair, blk, tcc)
                et = etp.tile([128, 2, 512], bf16, tag="et")
                nc.scalar.activation(et[:, :, :], psc[:, :, :], EXP, scale=0.125)
                ets.append(et)
                if prev is not None:
                    pv_mms(ph, prev[0], tcc, prev[2][tcc])
                bb = borrows.get(bi, [])
                if tcc == 4 and bi == 4 and len(bb) > 1:
                    proj_seg(kt if bb[1][0] == "k" else qt, bb[1][0], bb[1][1], bb[1][2])
                if tcc == 5 and bb:
                    proj_seg(kt if bb[0][0] == "k" else qt, bb[0][0], bb[0][1], bb[0][2])
                if tcc == 11 and len(bb) > 1 and bi != 4:
                    proj_seg(kt if bb[1][0] == "k" else qt, bb[1][0], bb[1][1], bb[1][2])
            if prev is not None:
                finalize(ph, prev[0], prev[1])
            prev = (pair, blk, ets)
        # drain: PV + finalize of the last block
        ph = alloc_ph()
        for tcc in range(SC):
            pv_mms(ph, prev[0], tcc, prev[2][tcc])
        finalize(ph, prev[0], prev[1])

        for p in (ps_ph, ps_qk, otp, hsp, etp, xbp, xfp, consts):
            p.release()

    nc.finalize()
    return nc


_NC = None


def _get_nc():
    global _NC
    if _NC is None:
        _NC = _build()
    return _NC


def _in_maps(inputs):
    x = np.asarray(inputs["hidden_states"], dtype=np.float32)
    m = np.asarray(inputs["attn_mask"], dtype=np.float32)
    wq = np.asarray(inputs["Wq"], dtype=np.float32)
    wk = np.asarray(inputs["Wk"], dtype=np.float32)
    wv = np.asarray(inputs["Wv"], dtype=np.float32)
    maps = []
    for c in range(NCORES):
        b, g = c // 4, c % 4
        sl = slice(g * WC, (g + 1) * WC)
        maps.append({
            "x": np.ascontiguousarray(x[b]),
            "m": np.ascontiguousarray(m[b]),
            "wq": np.ascontiguousarray(wq[sl]),
            "wk": np.ascontiguousarray(wk[sl]),
            "wv": np.ascontiguousarray(wv[sl]),
        })
    return maps


def _run(inputs, trace=False):
    from concourse.bass_utils import run_bass_kernel_spmd

    nc = _get_nc()
    res = run_bass_kernel_spmd(
        nc, _in_maps(inputs), core_ids=list(range(NCORES)), trace=trace
    )
    out = np.empty((B, S, D), dtype=np.float32)
    for c in range(NCORES):
        b, g = c // 4, c % 4
        out[b, :, g * WC:(g + 1) * WC] = res.results[c]["out"]
    return out, res


def kernel(**inputs):
    out, _ = _run(inputs, trace=False)
    return out


# revision 15
# speedup vs baseline: 1.6225x; 1.0853x over previous
"""Multi-head attention (B=2, S=2048, D=1024, 16 heads x 64) on 8 TRN2 cores.

Sharding: batch x head-group. Core c owns batch b = c//4 and head group
g = c%4 (4 heads, W-rows [256g, 256g+256)). Core output is the (2048, 256)
feature slice; host assembles [B, S, D]. No collectives.

Per-core pipeline (bf16 matmul operands, f32 PSUM):
  x, W: f32 load -> DVE cast bf16 -> PE transpose (packed 8/bank) -> xt/wt.
  q,k proj as [w, s] (lhsT = W.T chunk, rhs = xt). v proj as [s, w]
  (lhsT = xt chunk, rhs = Wv.T) -> v2[t, h, 65] with em[t]-scaled values
  and em[t] in column 64 (em = exp(1e4*mask - 1e4) folds the additive
  mask exactly; the 65th column makes PV also produce the softmax
  denominator Z). Projection biases are zeros by problem spec; skipped.
  Attention: 8 blocks (2 head pairs x 4 s-blocks of 512), software-
  pipelined one block deep: block k runs QK+exp while PV of block k-1
  consumes its stashed et tiles, so ACT (the bottleneck: 1024-row exp =
  ~1.11us, 128 calls = ~143us) never waits on PE.
  QK: two row-tiled K=64 matmuls, tile_position (0,0)/(64,0), run
  concurrently on the PE (measured 1.7x). PV in "swap" form: out[s=128,
  65] = et-chunk.T @ v2[t, 65] at 100% PE utilization (measured 32ns).
  Later q/k projection segments are interleaved into the attention loop
  ("borrows" of a psc PSUM slot) inside the PE slack.
  Finalize: DVE copies ph -> SBUF, GPSIMD divides by Z, HWDGE DMA out.
"""

import sys

if "/opt/trn_rl_repo" not in sys.path:
    sys.path.insert(0, "/opt/trn_rl_repo")

import numpy as np

B = 2
S = 2048
D = 1024
NCORES = 8
WC = 256          # per-core projection width (4 heads x 64)
NH = 4            # heads per core
NP = 2            # head pairs per core
W = 64            # head dim
KC = D // 128     # contraction chunks (8)
SC = S // 128     # 128-row chunks of S (16)
SEG = 512         # proj segment
NSEG = S // SEG   # 4
SBLK = 512        # attention s-block
NBLK = S // SBLK  # 4


def _build():
    import concourse.bass as bass
    import concourse.tile as tile
    from concourse import bacc, mybir
    from concourse.masks import make_identity

    f32 = mybir.dt.float32
    bf16 = mybir.dt.bfloat16
    EXP = mybir.ActivationFunctionType.Exp
    DIV = mybir.AluOpType.divide
    MUL = mybir.AluOpType.mult

    nc = bacc.Bacc("TRN2", target_bir_lowering=False, debug=False)

    x_d = nc.dram_tensor("x", [S, D], f32, kind="ExternalInput")
    m_d = nc.dram_tensor("m", [S], f32, kind="ExternalInput")
    wq_d = nc.dram_tensor("wq", [WC, D], f32, kind="ExternalInput")
    wk_d = nc.dram_tensor("wk", [WC, D], f32, kind="ExternalInput")
    wv_d = nc.dram_tensor("wv", [WC, D], f32, kind="ExternalInput")
    o_d = nc.dram_tensor("out", [S, WC], f32, kind="ExternalOutput")

    with tile.TileContext(nc) as tc:
        consts = tc.alloc_tile_pool(name="consts", bufs=1)
        xfp = tc.alloc_tile_pool(name="xfp", bufs=3)
        xbp = tc.alloc_tile_pool(name="xbp", bufs=3)
        etp = tc.alloc_tile_pool(name="etp", bufs=2 * SC)
        hsp = tc.alloc_tile_pool(name="hsp", bufs=4)
        otp = tc.alloc_tile_pool(name="otp", bufs=4)
        ps_qk = tc.alloc_tile_pool(name="ps_qk", bufs=2, space="PSUM")
        ps_ph = tc.alloc_tile_pool(name="ps_ph", bufs=4, space="PSUM")

        identb = consts.tile([128, 128], bf16, tag="identb")
        make_identity(nc, identb[:, :])

        # persistent SBUF tensors
        xt = consts.tile([128, KC, S], bf16, tag="xt")           # x.T
        wts = {n: consts.tile([128, KC, WC], bf16, tag=f"wt_{n}", name=f"wt_{n}")
               for n in ("q", "k", "v")}
        qt = consts.tile([128, NP, S], bf16, tag="qt")
        kt = consts.tile([128, NP, S], bf16, tag="kt")
        v2 = consts.tile([128, SC, NH, W + 1], bf16, tag="v2")
        em = consts.tile([128, SC], f32, tag="em")

        # --- weights + mask: DMA first ---
        wbufs = {}
        for i, (name, wd) in enumerate((("q", wq_d), ("k", wk_d), ("v", wv_d))):
            for blk in range(2):
                wf = xfp.tile([128, D], f32, tag="wf", name="wf", bufs=6)
                nc.gpsimd.dma_start(out=wf[:, :], in_=wd[blk * 128:(blk + 1) * 128, :])
                wbufs[(name, blk)] = wf

        msk = consts.tile([128, SC], f32, tag="msk")
        nc.gpsimd.dma_start(out=msk[:, :], in_=m_d.ap().rearrange("(c p) -> p c", p=128))
        mb = consts.tile([128, 1], f32, tag="mb")
        nc.vector.memset(mb[:, :], -10000.0)
        # em[t] = exp(1e4*mask - 1e4)  (1 for kept keys, ~0 for masked)
        nc.scalar.activation(em[:, :], msk[:, :], EXP, scale=10000.0, bias=mb[:, :])

        for name in ("q", "k", "v"):
            for blk in range(2):
                wf = wbufs[(name, blk)]
                wb = xbp.tile([128, D], bf16, tag="xb", name="wb")
                nc.vector.tensor_copy(wb[:, :], wf[:, :])
                pt_f = ps_qk.tile([128, 512], f32, tag="psc", name="wtr")
                pt = pt_f[:, :].bitcast(bf16).rearrange("p (a b) -> p a b", b=128)
                for kc in range(KC):
                    nc.tensor.transpose(pt[:, kc, :], wb[:, kc * 128:(kc + 1) * 128],
                                        identb[:, :])
                nc.vector.tensor_copy(wts[name][:, :, blk * 128:(blk + 1) * 128],
                                      pt[:, :, :])

        # v2 Z columns = em (bf16 cast)
        for h in range(NH):
            nc.vector.tensor_copy(
                v2[:, :, h, W:W + 1],
                em[:, :].rearrange("p (c one) -> p c one", one=1))

        # --- x: DMA (spread over 3 queues), cast, PE transpose into xt ---
        dma_engs = [nc.sync, nc.scalar, nc.gpsimd]
        xfs = []
        for sc in range(SC):
            xf = xfp.tile([128, D], f32, tag="xf", name="xf", bufs=6)
            dma_engs[sc % 3].dma_start(out=xf[:, :], in_=x_d[sc * 128:(sc + 1) * 128, :])
            xfs.append(xf)

        def xt_sc(sc):
            xb = xbp.tile([128, D], bf16, tag="xb", name="xb")
            nc.vector.tensor_copy(xb[:, :], xfs[sc][:, :])
            pt_f = ps_qk.tile([128, 512], f32, tag="psc", name="xtr")
            pt = pt_f[:, :].bitcast(bf16).rearrange("p (a b) -> p a b", b=128)
            for kc in range(KC):
                nc.tensor.transpose(pt[:, kc, :], xb[:, kc * 128:(kc + 1) * 128],
                                    identb[:, :])
            nc.vector.tensor_copy(xt[:, :, sc * 128:(sc + 1) * 128], pt[:, :, :])

        def proj_seg(dst, wname, pair, sseg):
            """dst[:, pair, sseg*512:...] = (W.T chunks @ xt) for one segment."""
            pp = ps_qk.tile([128, 512], f32, tag="psc", name="pp")
            wt = wts[wname]
            for kc in range(KC):
                nc.tensor.matmul(
                    pp[:, :],
                    lhsT=wt[:, kc, pair * 128:(pair + 1) * 128],
                    rhs=xt[:, kc, sseg * SEG:(sseg + 1) * SEG],
                    start=(kc == 0), stop=(kc == KC - 1),
                )
            nc.vector.tensor_copy(dst[:, pair, sseg * SEG:(sseg + 1) * SEG], pp[:, :])

        def vproj_sc(sc):
            """v2[:, sc, h, 0:64] = em[sc] * (x @ Wv.T)[sc-chunk] (as [s, w'])."""
            pv = ps_ph.tile([128, 512], f32, tag="ph", name="pv")
            for kc in range(KC):
                nc.tensor.matmul(
                    pv[:, 0:WC],
                    lhsT=xt[:, kc, sc * 128:(sc + 1) * 128],
                    rhs=wts["v"][:, kc, :],
                    start=(kc == 0), stop=(kc == KC - 1),
                )
            nc.vector.tensor_scalar(
                out=v2[:, sc, :, 0:W],
                in0=pv[:, 0:WC].rearrange("p (h w) -> p h w", h=NH),
                scalar1=em[:, sc:sc + 1], scalar2=None, op0=MUL,
            )

        # xt chunks with k-proj (pair 0) interleaved as segments complete,
        # then the first two q segments; keeps the PE queue fed in dep order
        for sc in range(SC):
            xt_sc(sc)
            if sc % 4 == 3:
                proj_seg(kt, "k", 0, sc // 4)
            if sc == 7:
                proj_seg(qt, "q", 0, 0)
        proj_seg(qt, "q", 0, 1)

        # --- attention: 8 blocks, PV pipelined one block behind ---
        def qk_mms(psc, pair, blk, tcc):
            for j in range(2):
                nc.tensor.matmul(
                    psc[:, j, :],
                    lhsT=kt[j * W:(j + 1) * W, pair, tcc * 128:(tcc + 1) * 128],
                    rhs=qt[j * W:(j + 1) * W, pair, blk * SBLK:(blk + 1) * SBLK],
                    start=True, stop=True,
                )

        def pv_mms(ph, pair, tcc, et):
            # start=False always: a start=True clears the WHOLE bank's
            # has_written bits, wiping the other head's region sharing the
            # bank. The banks are DVE-zeroed in alloc_ph instead; matmuls
            # then initialize-or-accumulate per element correctly.
            for j in range(2):
                h = pair * 2 + j
                for sc4 in range(4):
                    nc.tensor.matmul(
                        ph[sc4][:, j, 0:W + 1],
                        lhsT=et[:, j, sc4 * 128:(sc4 + 1) * 128],
                        rhs=v2[:, tcc, h, :],
                        start=(tcc == 0 and j == 0), stop=(tcc == SC - 1),
                        skip_group_check=True,
                    )

        def alloc_ph():
            # per-head stride 66 f32 (not 65): keeps the two heads'
            # accumulation regions on disjoint 8-byte PSUM cachelines
            # no memset needed: the first PV matmul per bank uses start=True,
            # whose whole-bank has_written clear makes every element's first
            # write an initialize (including the other head's region)
            ph_f = [ps_ph.tile([128, 512], f32, tag="ph", name="ph")
                    for _ in range(4)]
            return [p[:, 0:2 * (W + 2)].rearrange("p (h w) -> p h w", w=W + 2)
                    for p in ph_f]

        def finalize(ph, pair, blk):
            # h = ph[:, j, 0:64] / Z, Z = ph[:, j, 64]
            for sc4 in range(4):
                hsb = hsp.tile([128, 2, W + 2], f32, tag="hsb")
                nc.vector.tensor_copy(hsb[:, :, :], ph[sc4][:, :, :])
                rec = otp.tile([128, 2], f32, tag="rec")
                nc.vector.reciprocal(
                    rec[:, :], hsb[:, :, W:W + 1].rearrange("p h one -> p (h one)"))
                ot = otp.tile([128, 2 * W], f32, tag="ot")
                for j in range(2):
                    nc.vector.tensor_scalar(
                        out=ot[:, j * W:(j + 1) * W],
                        in0=hsb[:, j, 0:W],
                        scalar1=rec[:, j:j + 1],
                        scalar2=None, op0=MUL,
                    )
                s0 = blk * SBLK + sc4 * 128
                nc.sync.dma_start(
                    out=o_d[s0:s0 + 128, pair * 128:(pair + 1) * 128],
                    in_=ot[:, :])

        blocks = [(pair, blk) for pair in range(NP) for blk in range(NBLK)]
        # psc-slot borrows per block index (emitted at tcc 5 / 11; block 4's
        # kproj(1,3) at tcc 4 lands just before its tcc-12 QK needs it)
        borrows = {
            1: [("q", 0, 2), ("k", 1, 0)],
            2: [("q", 0, 3), ("k", 1, 1)],
            3: [("q", 1, 0), ("k", 1, 2)],
            4: [("q", 1, 1), ("k", 1, 3)],
            5: [("q", 1, 2)],
            6: [("q", 1, 3)],
        }
        prev = None
        for bi, (pair, blk) in enumerate(blocks):
            ph = alloc_ph() if prev is not None else None
            ets = []
            for tcc in range(SC):
                if bi == 0:
                    vproj_sc(tcc)
                psc = ps_qk.tile([128, 2, 512], f32, tag="psc", name="psc")
                qk_mms(psc, pair, blk, tcc)
                et = etp.tile([128, 2, 512], bf16, tag="et")
                nc.scalar.activation(et[:, :, :], psc[:, :, :], EXP, scale=0.125)
                ets.append(et)
                if prev is not None:
                    pv_mms(ph, prev[0], tcc, prev[2][tcc])
                bb = borrows.get(bi, [])
                if tcc == 4 and bi == 4 and len(bb) > 1:
                    proj_seg(kt if bb[1][0] == "k" else qt, bb[1][0], bb[1][1], bb[1][2])
                if tcc == 5 and bb:
                    proj_seg(kt if bb[0][0] == "k" else qt, bb[0][0], bb[0][1], bb[0][2])
                if tcc == 11 and len(bb) > 1 and bi != 4:
                    proj_seg(kt if bb[1][0] == "k" else qt, bb[1][0], bb[1][1], bb[1][2])
            if prev is not None:
                finalize(ph, prev[0], prev[1])
            prev = (pair, blk, ets)
        # drain: PV + finalize of the last block
        ph = alloc_ph()
        for tcc in range(SC):
            pv_mms(ph, prev[0], tcc, prev[2][tcc])
        finalize(ph, prev[0], prev[1])

        for p in (ps_ph, ps_qk, otp, hsp, etp, xbp, xfp, consts):
            p.release()

    nc.finalize()
    return nc


_NC = None


def _get_nc():
    global _NC
    if _NC is None:
        _NC = _build()
    return _NC


def _in_maps(inputs):
    x = np.asarray(inputs["hidden_states"], dtype=np.float32)
    m = np.asarray(inputs["attn_mask"], dtype=np.float32)
    wq = np.asarray(inputs["Wq"], dtype=np.float32)
    wk = np.asarray(inputs["Wk"], dtype=np.float32)
    wv = np.asarray(inputs["Wv"], dtype=np.float32)
    maps = []
    for c in range(NCORES):
        b, g = c // 4, c % 4
        sl = slice(g * WC, (g + 1) * WC)
        maps.append({
            "x": np.ascontiguousarray(x[b]),
            "m": np.ascontiguousarray(m[b]),
            "wq": np.ascontiguousarray(wq[sl]),
            "wk": np.ascontiguousarray(wk[sl]),
            "wv": np.ascontiguousarray(wv[sl]),
        })
    return maps


def _run(inputs, trace=False):
    from concourse.bass_utils import run_bass_kernel_spmd

    nc = _get_nc()
    res = run_bass_kernel_spmd(
        nc, _in_maps(inputs), core_ids=list(range(NCORES)), trace=trace
    )
    out = np.empty((B, S, D), dtype=np.float32)
    for c in range(NCORES):
        b, g = c // 4, c % 4
        out[b, :, g * WC:(g + 1) * WC] = res.results[c]["out"]
    return out, res


def kernel(**inputs):
    out, _ = _run(inputs, trace=False)
    return out
